# revision 30
# baseline (speedup 1.0000x reference)
"""Point spatial attention (offset-attention) Trainium2 kernel.

Data-parallel over batch B=8 across 8 NeuronCores; each core runs one
point cloud (N=4096) end-to-end:

  feat = w2 @ relu(bn1(w1 @ (x+offset)))          [128, N]
  q/k/v = relu(bn(w @ feat))                      [16/16/3, N]
  energy = q^T k                                  [N, N]
  sim = softmax_row(energy); sim /= colsum(sim)
  out = alpha * (v @ sim) + x                     [3, N]

Device algorithm (single pass over the [N, N] matrix, ~135 us/core by
the instruction cost model):
  - BN affines folded into conv weights host-side; w2 is folded into
    the q/k/v weights too (no nonlinearity between them), so the head
    is just two small matmul stages.
  - All matmul operands in bf16 (fp32 moving operands stream at 1/4
    rate on the PE); accumulation stays fp32 in PSUM.  Energies are
    ~0.04 and the near-uniform softmax averages the bf16 rounding away
    (measured 3e-9 scale-relative final error vs the f32 reference).
  - Softmax without max-subtraction (energy in [0, 0.08]; exp of that
    range is exact-safe in f32).
  - Per 128-row block i: E_i = exp(q_i^T k), split between the ACT
    engine (ACTIVATE Exp, row-sum fused via accum_out) and the DVE (a
    custom fused op computing a quadratic fit of exp + accumulate in
    one pass) so both engines share the N^2 exp bottleneck.  Then
    v'_i = [v; 1]^T / rowsum and numer += v'_i^T E_i accumulates in
    PSUM across all 32 blocks, one block behind the exp pipeline so
    the PE never starves the exp engines.  The extra ones-row of v'
    yields colsum(sim), making the final column normalization a
    reciprocal+multiply at the end.
  - numer PSUM lives in 2 banks: 8 m-chunks of [4, 512] packed at
    partition offsets 0/32/64/96 via tensor-engine column tiling,
    leaving 6 banks for triple-buffered energy/exp chunks.
"""

import time
from contextlib import ExitStack

import numpy as np

import concourse.bass as bass
import concourse.mybir as mybir
import concourse.tile as tile
from concourse import bacc
from concourse.bass_utils import run_bass_kernel_spmd
from concourse.masks import make_identity


def _register_exp_poly():
    """Fused quadratic-poly exp with row-sum accumulate, one DVE pass:
    out = ((x + s0) * x) * s1 + imm2;  accum_out = sum(out).
    Registered at import into dve_ops.OPS (runtime append, row 17+)."""
    from operator import add as _add
    import concourse.dve_ops as dve_ops
    from concourse.dve_spec import Spec, Src0, C0, C1, C2, lower
    from concourse.dve_uop import DveOpSpec
    from concourse.dve_table_gen import dve_ver_for

    name = "EXP_POLY_ACC_ANT"
    if name in dve_ops._SUB_OPCODE_FOR_NAME:
        return next(op for op in dve_ops.OPS if op.name == name)

    def _ref(in0, in1, c0, c1, c2):
        b = (((in0.astype(np.float32) + c0) * in0) * c1 + c2).astype(np.float32)
        return b, b.reshape(b.shape[0], -1).sum(axis=-1, keepdims=True)

    spec = Spec(body=((Src0 + C0) * Src0) * C1 + C2, accum=_add, reference=_ref)
    row = dve_ops._CUSTOM_DVE_ROW_BASE + len(dve_ops.OPS)
    assert row < 0x20
    shas = {}
    for ver in ("v3", "v4"):
        ds = DveOpSpec(name=name, opcode=row, uops=lower(spec, ver=ver),
                       rd1_en=False)
        shas[ver] = ds.sha(ver)
    op = dve_ops.DveOp(name, spec, subdim=False, uops_sha=shas)
    dve_ops.OPS.append(op)
    dve_ops._SUB_OPCODE_FOR_NAME[name] = row
    dve_ops.CUSTOM_DVE_SPECS[name] = spec
    return op


EXP_POLY = _register_exp_poly()

F32 = mybir.dt.float32
BF16 = mybir.dt.bfloat16
BN_EPS = 1e-5
N = 4096
B = 8
N_CORES = 8
P = 128


def _chunks(total, maxc):
    out = []
    rem = total
    while rem > 0:
        c = min(maxc, rem)
        out.append((total - rem, c))
        rem -= c
    return out


def build_program(n=N, n_cores=N_CORES):
    nc = bacc.Bacc("TRN2", target_bir_lowering=False, debug=False,
                   num_devices=n_cores)
    nb = n // P           # row blocks
    n_mch = n // 512      # m-chunks for the numer matmuls (<= 8)
    n_banks = (n_mch + 3) // 4   # numer psum banks
    if n >= 4096:
        # (offset, len, engine): ACT does exp, DVE does the fused poly-exp
        ech = [(0, 1024, "A"), (1024, 1024, "A"),
               (2048, 1024, "D"), (3072, 1024, "D")]
    else:
        ech = [(off, ln, ("D" if len(_chunks(n, 1536)) >= 2
                          and i == len(_chunks(n, 1536)) - 1 else "A"))
               for i, (off, ln) in enumerate(_chunks(n, 1536))]
    assert n_mch <= 8 and n % 1024 == 0 and all(c[1] <= 1024 for c in ech)

    x4_d = nc.dram_tensor("x4", [P, n_banks, 512], F32, kind="ExternalInput")
    xbf_d = nc.dram_tensor("xbf", [3, n], BF16, kind="ExternalInput")
    w1t_d = nc.dram_tensor("w1t", [3, 64], BF16, kind="ExternalInput")
    t1_d = nc.dram_tensor("t1", [64, 1], F32, kind="ExternalInput")
    wqkvt_d = nc.dram_tensor("wqkvt", [64, 35], BF16, kind="ExternalInput")
    tqkv_d = nc.dram_tensor("tqkv", [35, 1], F32, kind="ExternalInput")
    alpha_d = nc.dram_tensor("alphav", [P, 1], F32, kind="ExternalInput")
    out_d = nc.dram_tensor("out", [3, n], F32, kind="ExternalOutput")

    AL = mybir.AluOpType
    Exp = mybir.ActivationFunctionType.Exp
    Relu = mybir.ActivationFunctionType.Relu
    Ident = mybir.ActivationFunctionType.Identity
    AX = mybir.AxisListType.X

    with ExitStack() as ctx:
        tc = ctx.enter_context(tile.TileContext(nc))
        consts = ctx.enter_context(tc.tile_pool(name="consts", bufs=1))
        sb = ctx.enter_context(tc.tile_pool(name="sb", bufs=1))
        epool = ctx.enter_context(tc.tile_pool(name="epsum", bufs=3, space="PSUM"))
        npool = ctx.enter_context(tc.tile_pool(name="npsum", bufs=1, space="PSUM"))
        Epool = ctx.enter_context(tc.tile_pool(name="Esb", bufs=3))
        small = ctx.enter_context(tc.tile_pool(name="small", bufs=4))

        # ---- constant loads (weights first; tail-only tensors later) ----
        w1t = consts.tile([3, 64], BF16)
        nc.sync.dma_start(w1t[:], w1t_d.ap()[:])
        t1 = consts.tile([64, 1], F32)
        nc.sync.dma_start(t1[:], t1_d.ap()[:])
        wqkvt = consts.tile([64, 35], BF16)
        nc.gpsimd.dma_start(wqkvt[:], wqkvt_d.ap()[:])
        tqkv = consts.tile([35, 1], F32)
        nc.gpsimd.dma_start(tqkv[:], tqkv_d.ap()[:])
        xbf_sb = consts.tile([3, n], BF16)
        for c in range(n // 1024):
            sl = slice(c * 1024, (c + 1) * 1024)
            nc.sync.dma_start(xbf_sb[:, sl], xbf_d.ap()[:, sl])
        x4_sb = consts.tile([P, n_banks, 512], F32)
        nc.gpsimd.dma_start(x4_sb[:], x4_d.ap()[:])
        alphav = consts.tile([P, 1], F32)
        nc.gpsimd.dma_start(alphav[:], alpha_d.ap()[:])

        # ---- head (chunk-interleaved):
        #   r1 = relu(w1' x + t1')  [DVE]
        #   qkv = relu((Wqkv w2) r1 + tqkv)  [ACT]; rows 0-15 q, 16-31 k, 32-34 v
        # (w2 is folded into the qkv weights host-side -- no feat stage)
        r1_sb = sb.tile([64, n], BF16)
        qkv_sb = sb.tile([35, n], BF16)
        k_sb = sb.tile([16, n], BF16)
        v_sb = sb.tile([3, n], BF16)
        ident = consts.tile([3, 3], BF16)
        make_identity(nc, ident)
        for c in range(n // 1024):
            ch = slice(c * 1024, (c + 1) * 1024)
            h1 = epool.tile([P, 1024], F32, tag="e")
            for s in range(2):
                sl = slice(c * 1024 + s * 512, c * 1024 + (s + 1) * 512)
                nc.tensor.matmul(h1[0:64, s * 512:(s + 1) * 512],
                                 w1t[:], xbf_sb[:, sl], start=True, stop=True)
            nc.vector.tensor_scalar(
                out=r1_sb[:, ch], in0=h1[0:64, 0:1024],
                scalar1=t1[:], scalar2=0.0, op0=AL.add, op1=AL.max)
            qp = epool.tile([P, 1024], F32, tag="e")
            for s in range(2):
                sl = slice(c * 1024 + s * 512, c * 1024 + (s + 1) * 512)
                nc.tensor.matmul(qp[0:35, s * 512:(s + 1) * 512],
                                 wqkvt[:], r1_sb[:, sl], start=True, stop=True)
            nc.scalar.activation(
                out=qkv_sb[:, ch], in_=qp[0:35, 0:1024],
                func=Relu, bias=tqkv[:], scale=1.0)
            # k/v shifted to base partition 0 as each chunk lands
            nc.sync.dma_start(k_sb[:, ch], qkv_sb[16:32, ch])
            nc.gpsimd.dma_start(v_sb[:, ch], qkv_sb[32:35, ch])
        # v transposes (batched after the loop; v chunks landed during it)
        assert 4 * nb <= 2048
        tp = epool.tile([P, 2048], BF16, tag="e", name="tp")
        for i in range(nb):
            nc.tensor.transpose(tp[:, 4 * i:4 * i + 3],
                                v_sb[:, i * P:(i + 1) * P], ident[:])

        # vT_ext [128, nb, 4] bf16: cols 0-2 = v^T, col 3 = 1.0 (colsum
        # carrier); the per-chunk transposes above landed in tp
        vT = sb.tile([P, nb, 4], BF16)
        nc.vector.memset(vT[:], 1.0)
        tp4 = tp[:, 0:4 * nb].rearrange("p (a b) -> p a b", b=4)
        nc.vector.tensor_copy(vT[:, :, 0:3], tp4[:, :, 0:3])

        # numer accumulators: m-chunk j -> bank j//4, partitions 32*(j%4)+0..3
        numer_ps = []
        for bk in range(n_banks):
            nt = npool.tile([P, 512], F32, tag=f"numer{bk}", name=f"numer{bk}")
            nc.vector.memset(nt[:], 0.0)
            numer_ps.append(nt)

        # quadratic Chebyshev fit of exp on [0, 0.25] for the DVE-side exp
        # (energies are >= 0 since q,k are post-relu; observed max ~0.073,
        # fit error ~1e-5 -- far below the bf16 storage rounding of E):
        # exp(x) ~= c2*x^2 + c1*x + c0 = ((x + c1/c2) * x) * c2 + c0
        _xs = np.cos(np.pi * (np.arange(64) + 0.5) / 64) * 0.125 + 0.125
        _cf = np.polyfit(_xs, np.exp(_xs), 2)
        PC2, PC1, PC0 = float(_cf[0]), float(_cf[1]), float(_cf[2])

        # ---- main loop over row blocks ----
        pending = []
        for i in range(nb):
            E_sb = Epool.tile([P, n], BF16, tag="E")
            racc = small.tile([P, max(len(ech), 2)], F32, tag="racc")
            for ci, (off, ln, eng) in enumerate(ech):
                e_ps = epool.tile([P, 1024], F32, tag="e")
                for s in range(0, ln, 512):
                    sl = slice(off + s, off + s + 512)
                    nc.tensor.matmul(e_ps[:, s:s + 512],
                                     qkv_sb[0:16, i * P:(i + 1) * P],
                                     k_sb[:, sl], start=True, stop=True)
                if eng == "D":
                    nc.vector._custom_dve(
                        EXP_POLY, out=E_sb[:, off:off + ln],
                        in0=e_ps[:, 0:ln], s0=PC1 / PC2, s1=PC2, imm2=PC0,
                        accum_out=racc[:, ci:ci + 1])
                else:
                    nc.scalar.activation(
                        out=E_sb[:, off:off + ln], in_=e_ps[:, 0:ln],
                        func=Exp, accum_out=racc[:, ci:ci + 1])
            rs = small.tile([P, 1], F32, tag="rs")
            nc.vector.reduce_sum(rs[:], racc[:, 0:len(ech)], axis=AX)
            inv = small.tile([P, 1], F32, tag="inv")
            nc.vector.reciprocal(inv[:], rs[:])
            vp = small.tile([P, 4], BF16, tag="vp")
            nc.gpsimd.tensor_scalar_mul(vp[:], vT[:, i, :], inv[:])
            pending.append((vp, E_sb))
            # numer matmuls run one block behind so the PE never starves the
            # ACT/DVE exp of the current block
            if len(pending) > 1:
                pvp, pE = pending.pop(0)
                ip = i - 1
                for j in range(n_mch):
                    jj, bk = j % 4, j // 4
                    nc.tensor.matmul(
                        numer_ps[bk][32 * jj:32 * jj + 4, :], pvp[:],
                        pE[:, j * 512:(j + 1) * 512],
                        start=(ip == 0), stop=False,
                        tile_position=(0, 32 * jj))

        # drain the last pending block's numer matmuls
        pvp, pE = pending.pop(0)
        for j in range(n_mch):
            jj, bk = j % 4, j // 4
            nc.tensor.matmul(
                numer_ps[bk][32 * jj:32 * jj + 4, :], pvp[:],
                pE[:, j * 512:(j + 1) * 512],
                start=(nb == 1), stop=True,
                tile_position=(0, 32 * jj))

        # ---- final: out = alpha * numer/(1e-9+colsum) + x ----
        epsb = consts.tile([P, 1], F32)
        nc.vector.memset(epsb[:], 1e-9)
        recip_sb = sb.tile([P, n_banks, 512], F32)
        rep_sb = sb.tile([P, n_banks, 512], F32)
        nc.vector.memset(rep_sb[:], 0.0)
        for bk in range(n_banks):
            nc.scalar.activation(out=recip_sb[:, bk, :], in_=numer_ps[bk][:],
                                 func=Ident, bias=epsb[:], scale=1.0)
            nc.vector.reciprocal(recip_sb[:, bk, :], recip_sb[:, bk, :])
        for j in range(n_mch):
            jj, bk = j % 4, j // 4
            src = recip_sb[32 * jj + 3:32 * jj + 4, bk, :]
            # free-dim step-0 broadcast: re-read the same 512 row 4x while
            # the dst walks 4 partitions (partition step 0 is not allowed)
            src_b = bass.AP(tensor=src.tensor, offset=src.offset,
                            ap=[list(src.ap[0]), [0, 4], list(src.ap[-1])])
            qs3 = (nc.gpsimd, nc.sync, nc.scalar)
            qs3[j % 3].dma_start(rep_sb[32 * jj:32 * jj + 4, bk, :], src_b)
        att = sb.tile([P, n_banks, 512], F32)
        for bk in range(n_banks):
            nc.vector.tensor_mul(att[:, bk, :], numer_ps[bk][:], rep_sb[:, bk, :])
        # out = alpha*att + x, computed in the scattered numer layout (x4 is
        # host-prepared in the same layout), then DMA'd straight to DRAM
        out_sc = sb.tile([P, n_banks, 512], F32)
        qs = (nc.sync, nc.gpsimd, nc.scalar)
        for bk in range(n_banks):
            nc.vector.scalar_tensor_tensor(
                out=out_sc[:, bk, :], in0=att[:, bk, :], scalar=alphav[:],
                in1=x4_sb[:, bk, :], op0=AL.mult, op1=AL.add)
            for jj in range(min(4, n_mch - 4 * bk)):
                j = 4 * bk + jj
                qs[j % 3].dma_start(out_d.ap()[:, j * 512:(j + 1) * 512],
                                    out_sc[32 * jj:32 * jj + 3, bk, :])

    nc.compile()
    return nc


def fold_weights(inputs):
    """Host-side BN folding. Returns the per-core constant input dict."""
    import ml_dtypes
    bf16 = ml_dtypes.bfloat16

    def fold(w, g, b, m, v):
        s = (g / np.sqrt(v + BN_EPS)).astype(np.float64)
        t = b.astype(np.float64) - s * m.astype(np.float64)
        return s[:, None] * w.astype(np.float64), t

    w1p, t1 = fold(inputs["w1"], inputs["g1"], inputs["b1"],
                   inputs["m1"], inputs["v1"])
    t1 = t1 + float(np.asarray(inputs["offset"]).ravel()[0]) * w1p.sum(axis=1)
    wqp, tq = fold(inputs["wq"], inputs["gq"], inputs["bq"],
                   inputs["mq"], inputs["vq"])
    wkp, tk = fold(inputs["wk"], inputs["gk"], inputs["bk"],
                   inputs["mk"], inputs["vk"])
    wvp, tv = fold(inputs["wv"], inputs["gv"], inputs["bv"],
                   inputs["mv"], inputs["vv"])
    w2 = np.asarray(inputs["w2"]).astype(np.float64)
    wqkv = np.concatenate([wqp, wkp, wvp], axis=0) @ w2   # [35, 64]
    tqkv = np.concatenate([tq, tk, tv], axis=0)           # [35]
    alpha = float(np.asarray(inputs["alpha"]).ravel()[0])
    return {
        "w1t": np.ascontiguousarray(w1p.T).astype(bf16),
        "t1": t1.astype(np.float32).reshape(64, 1),
        "wqkvt": np.ascontiguousarray(wqkv.T).astype(bf16),
        "tqkv": tqkv.astype(np.float32).reshape(35, 1),
        "alphav": np.full((128, 1), alpha, np.float32),
    }


_prog_cache = {}


def get_program(n=N, n_cores=N_CORES):
    key = (n, n_cores)
    if key not in _prog_cache:
        _prog_cache[key] = build_program(n, n_cores)
    return _prog_cache[key]


def make_x4(xb, n=N):
    """Scatter x [3, n] into the numer psum layout [128, n_banks, 512]."""
    n_mch = n // 512
    n_banks = (n_mch + 3) // 4
    x4 = np.zeros((128, n_banks, 512), np.float32)
    for j in range(n_mch):
        jj, bk = j % 4, j // 4
        x4[32 * jj:32 * jj + 3, bk, :] = xb[:, j * 512:(j + 1) * 512]
    return x4


def kernel(_trace=False, _trace_kwargs=None, **inputs):
    import ml_dtypes
    inputs = {k: np.asarray(v) for k, v in inputs.items()}
    nc = get_program()
    const_ins = fold_weights(inputs)
    x = inputs["x"].astype(np.float32)
    in_maps = [dict(const_ins,
                    x4=make_x4(x[b]),
                    xbf=np.ascontiguousarray(x[b]).astype(ml_dtypes.bfloat16))
               for b in range(B)]
    res = run_bass_kernel_spmd(nc, in_maps, core_ids=list(range(N_CORES)),
                               trace=_trace, **(_trace_kwargs or {}))
    out = np.stack([res.results[b]["out"] for b in range(B)], axis=0)
    if _trace:
        kernel.last_result = res
    return out.astype(np.float32)


if __name__ == "__main__":
    t0 = time.time()
    nc = get_program()
    print("build+compile:", time.time() - t0, flush=True)


# revision 31
# speedup vs baseline: 1.0048x; 1.0048x over previous
"""Point spatial attention (offset-attention) Trainium2 kernel.

Data-parallel over batch B=8 across 8 NeuronCores; each core runs one
point cloud (N=4096) end-to-end:

  feat = w2 @ relu(bn1(w1 @ (x+offset)))          [128, N]
  q/k/v = relu(bn(w @ feat))                      [16/16/3, N]
  energy = q^T k                                  [N, N]
  sim = softmax_row(energy); sim /= colsum(sim)
  out = alpha * (v @ sim) + x                     [3, N]

Device algorithm (single pass over the [N, N] matrix, ~135 us/core by
the instruction cost model):
  - BN affines folded into conv weights host-side; w2 is folded into
    the q/k/v weights too (no nonlinearity between them), so the head
    is just two small matmul stages.
  - All matmul operands in bf16 (fp32 moving operands stream at 1/4
    rate on the PE); accumulation stays fp32 in PSUM.  Energies are
    ~0.04 and the near-uniform softmax averages the bf16 rounding away
    (measured 3e-9 scale-relative final error vs the f32 reference).
  - Softmax without max-subtraction (energy in [0, 0.08]; exp of that
    range is exact-safe in f32).
  - Per 128-row block i: E_i = exp(q_i^T k), split between the ACT
    engine (ACTIVATE Exp, row-sum fused via accum_out) and the DVE (a
    custom fused op computing a quadratic fit of exp + accumulate in
    one pass) so both engines share the N^2 exp bottleneck.  Then
    v'_i = [v; 1]^T / rowsum and numer += v'_i^T E_i accumulates in
    PSUM across all 32 blocks, one block behind the exp pipeline so
    the PE never starves the exp engines.  The extra ones-row of v'
    yields colsum(sim), making the final column normalization a
    reciprocal+multiply at the end.
  - numer PSUM lives in 2 banks: 8 m-chunks of [4, 512] packed at
    partition offsets 0/32/64/96 via tensor-engine column tiling,
    leaving 6 banks for triple-buffered energy/exp chunks.
"""

import time
from contextlib import ExitStack

import numpy as np

import concourse.bass as bass
import concourse.mybir as mybir
import concourse.tile as tile
from concourse import bacc
from concourse.bass_utils import run_bass_kernel_spmd
from concourse.masks import make_identity


def _register_exp_poly():
    """Fused quadratic-poly exp with row-sum accumulate, one DVE pass:
    out = ((x + s0) * x) * s1 + imm2;  accum_out = sum(out).
    Registered at import into dve_ops.OPS (runtime append, row 17+)."""
    from operator import add as _add
    import concourse.dve_ops as dve_ops
    from concourse.dve_spec import Spec, Src0, C0, C1, C2, lower
    from concourse.dve_uop import DveOpSpec
    from concourse.dve_table_gen import dve_ver_for

    name = "EXP_POLY_ACC_ANT"
    if name in dve_ops._SUB_OPCODE_FOR_NAME:
        return next(op for op in dve_ops.OPS if op.name == name)

    def _ref(in0, in1, c0, c1, c2):
        b = (((in0.astype(np.float32) + c0) * in0) * c1 + c2).astype(np.float32)
        return b, b.reshape(b.shape[0], -1).sum(axis=-1, keepdims=True)

    spec = Spec(body=((Src0 + C0) * Src0) * C1 + C2, accum=_add, reference=_ref)
    row = dve_ops._CUSTOM_DVE_ROW_BASE + len(dve_ops.OPS)
    assert row < 0x20
    shas = {}
    for ver in ("v3", "v4"):
        ds = DveOpSpec(name=name, opcode=row, uops=lower(spec, ver=ver),
                       rd1_en=False)
        shas[ver] = ds.sha(ver)
    op = dve_ops.DveOp(name, spec, subdim=False, uops_sha=shas)
    dve_ops.OPS.append(op)
    dve_ops._SUB_OPCODE_FOR_NAME[name] = row
    dve_ops.CUSTOM_DVE_SPECS[name] = spec
    return op


EXP_POLY = _register_exp_poly()

F32 = mybir.dt.float32
BF16 = mybir.dt.bfloat16
BN_EPS = 1e-5
N = 4096
B = 8
N_CORES = 8
P = 128


def _chunks(total, maxc):
    out = []
    rem = total
    while rem > 0:
        c = min(maxc, rem)
        out.append((total - rem, c))
        rem -= c
    return out


def build_program(n=N, n_cores=N_CORES):
    nc = bacc.Bacc("TRN2", target_bir_lowering=False, debug=False,
                   num_devices=n_cores)
    nb = n // P           # row blocks
    n_mch = n // 512      # m-chunks for the numer matmuls (<= 8)
    n_banks = (n_mch + 3) // 4   # numer psum banks
    if n >= 4096:
        # (offset, len, engine): ACT does exp, DVE does the fused poly-exp
        ech = [(0, 1024, "A"), (1024, 1024, "A"),
               (2048, 1024, "D"), (3072, 1024, "D")]
    else:
        ech = [(off, ln, ("D" if len(_chunks(n, 1536)) >= 2
                          and i == len(_chunks(n, 1536)) - 1 else "A"))
               for i, (off, ln) in enumerate(_chunks(n, 1536))]
    assert n_mch <= 8 and n % 1024 == 0 and all(c[1] <= 1024 for c in ech)

    x4_d = nc.dram_tensor("x4", [P, n_banks, 512], F32, kind="ExternalInput")
    xbf_d = nc.dram_tensor("xbf", [3, n], BF16, kind="ExternalInput")
    w1t_d = nc.dram_tensor("w1t", [3, 64], BF16, kind="ExternalInput")
    t1_d = nc.dram_tensor("t1", [64, 1], F32, kind="ExternalInput")
    wqkvt_d = nc.dram_tensor("wqkvt", [64, 35], BF16, kind="ExternalInput")
    tqkv_d = nc.dram_tensor("tqkv", [35, 1], F32, kind="ExternalInput")
    alpha_d = nc.dram_tensor("alphav", [P, 1], F32, kind="ExternalInput")
    out_d = nc.dram_tensor("out", [3, n], F32, kind="ExternalOutput")

    AL = mybir.AluOpType
    Exp = mybir.ActivationFunctionType.Exp
    Relu = mybir.ActivationFunctionType.Relu
    Ident = mybir.ActivationFunctionType.Identity
    AX = mybir.AxisListType.X

    with ExitStack() as ctx:
        tc = ctx.enter_context(tile.TileContext(nc))
        consts = ctx.enter_context(tc.tile_pool(name="consts", bufs=1))
        sb = ctx.enter_context(tc.tile_pool(name="sb", bufs=1))
        epool = ctx.enter_context(tc.tile_pool(name="epsum", bufs=3, space="PSUM"))
        npool = ctx.enter_context(tc.tile_pool(name="npsum", bufs=1, space="PSUM"))
        Epool = ctx.enter_context(tc.tile_pool(name="Esb", bufs=3))
        small = ctx.enter_context(tc.tile_pool(name="small", bufs=4))

        # ---- constant loads (weights first; tail-only tensors later) ----
        w1t = consts.tile([3, 64], BF16)
        nc.sync.dma_start(w1t[:], w1t_d.ap()[:])
        t1 = consts.tile([64, 1], F32)
        nc.sync.dma_start(t1[:], t1_d.ap()[:])
        wqkvt = consts.tile([64, 35], BF16)
        nc.gpsimd.dma_start(wqkvt[:], wqkvt_d.ap()[:])
        tqkv = consts.tile([35, 1], F32)
        nc.gpsimd.dma_start(tqkv[:], tqkv_d.ap()[:])
        xbf_sb = consts.tile([3, n], BF16)
        for c in range(n // 1024):
            sl = slice(c * 1024, (c + 1) * 1024)
            nc.sync.dma_start(xbf_sb[:, sl], xbf_d.ap()[:, sl])
        x4_sb = consts.tile([P, n_banks, 512], F32)
        nc.gpsimd.dma_start(x4_sb[:], x4_d.ap()[:])
        alphav = consts.tile([P, 1], F32)
        nc.gpsimd.dma_start(alphav[:], alpha_d.ap()[:])

        # ---- head (chunk-interleaved):
        #   r1 = relu(w1' x + t1')  [DVE]
        #   qkv = relu((Wqkv w2) r1 + tqkv)  [ACT]; rows 0-15 q, 16-31 k, 32-34 v
        # (w2 is folded into the qkv weights host-side -- no feat stage)
        r1_sb = sb.tile([64, n], BF16)
        qkv_sb = sb.tile([35, n], BF16)
        k_sb = sb.tile([16, n], BF16)
        v_sb = sb.tile([3, n], BF16)
        ident = consts.tile([3, 3], BF16)
        make_identity(nc, ident)
        for c in range(n // 1024):
            ch = slice(c * 1024, (c + 1) * 1024)
            h1 = epool.tile([P, 1024], F32, tag="e")
            for s in range(2):
                sl = slice(c * 1024 + s * 512, c * 1024 + (s + 1) * 512)
                nc.tensor.matmul(h1[0:64, s * 512:(s + 1) * 512],
                                 w1t[:], xbf_sb[:, sl], start=True, stop=True)
            for s in range(2):
                nc.vector.tensor_scalar(
                    out=r1_sb[:, c * 1024 + s * 512:c * 1024 + (s + 1) * 512],
                    in0=h1[0:64, s * 512:(s + 1) * 512],
                    scalar1=t1[:], scalar2=0.0, op0=AL.add, op1=AL.max)
            qp = epool.tile([P, 1024], F32, tag="e")
            for s in range(2):
                sl = slice(c * 1024 + s * 512, c * 1024 + (s + 1) * 512)
                nc.tensor.matmul(qp[0:35, s * 512:(s + 1) * 512],
                                 wqkvt[:], r1_sb[:, sl], start=True, stop=True)
            nc.scalar.activation(
                out=qkv_sb[:, ch], in_=qp[0:35, 0:1024],
                func=Relu, bias=tqkv[:], scale=1.0)
            # k/v shifted to base partition 0 as each chunk lands
            nc.sync.dma_start(k_sb[:, ch], qkv_sb[16:32, ch])
            nc.gpsimd.dma_start(v_sb[:, ch], qkv_sb[32:35, ch])
        # v transposes (batched after the loop; v chunks landed during it)
        assert 4 * nb <= 2048
        tp = epool.tile([P, 2048], BF16, tag="e", name="tp")
        for i in range(nb):
            nc.tensor.transpose(tp[:, 4 * i:4 * i + 3],
                                v_sb[:, i * P:(i + 1) * P], ident[:])

        # vT_ext [128, nb, 4] bf16: cols 0-2 = v^T, col 3 = 1.0 (colsum
        # carrier); the per-chunk transposes above landed in tp
        vT = sb.tile([P, nb, 4], BF16)
        nc.vector.memset(vT[:], 1.0)
        tp4 = tp[:, 0:4 * nb].rearrange("p (a b) -> p a b", b=4)
        nc.vector.tensor_copy(vT[:, :, 0:3], tp4[:, :, 0:3])

        # numer accumulators: m-chunk j -> bank j//4, partitions 32*(j%4)+0..3
        numer_ps = []
        for bk in range(n_banks):
            nt = npool.tile([P, 512], F32, tag=f"numer{bk}", name=f"numer{bk}")
            nc.vector.memset(nt[:], 0.0)
            numer_ps.append(nt)

        # quadratic Chebyshev fit of exp on [0, 0.25] for the DVE-side exp
        # (energies are >= 0 since q,k are post-relu; observed max ~0.073,
        # fit error ~1e-5 -- far below the bf16 storage rounding of E):
        # exp(x) ~= c2*x^2 + c1*x + c0 = ((x + c1/c2) * x) * c2 + c0
        _xs = np.cos(np.pi * (np.arange(64) + 0.5) / 64) * 0.125 + 0.125
        _cf = np.polyfit(_xs, np.exp(_xs), 2)
        PC2, PC1, PC0 = float(_cf[0]), float(_cf[1]), float(_cf[2])

        # ---- main loop over row blocks ----
        pending = []
        for i in range(nb):
            E_sb = Epool.tile([P, n], BF16, tag="E")
            racc = small.tile([P, max(len(ech), 2)], F32, tag="racc")
            for ci, (off, ln, eng) in enumerate(ech):
                e_ps = epool.tile([P, 1024], F32, tag="e")
                for s in range(0, ln, 512):
                    sl = slice(off + s, off + s + 512)
                    nc.tensor.matmul(e_ps[:, s:s + 512],
                                     qkv_sb[0:16, i * P:(i + 1) * P],
                                     k_sb[:, sl], start=True, stop=True)
                if eng == "D":
                    nc.vector._custom_dve(
                        EXP_POLY, out=E_sb[:, off:off + ln],
                        in0=e_ps[:, 0:ln], s0=PC1 / PC2, s1=PC2, imm2=PC0,
                        accum_out=racc[:, ci:ci + 1])
                else:
                    nc.scalar.activation(
                        out=E_sb[:, off:off + ln], in_=e_ps[:, 0:ln],
                        func=Exp, accum_out=racc[:, ci:ci + 1])
            rs = small.tile([P, 1], F32, tag="rs")
            nc.vector.reduce_sum(rs[:], racc[:, 0:len(ech)], axis=AX)
            inv = small.tile([P, 1], F32, tag="inv")
            nc.vector.reciprocal(inv[:], rs[:])
            vp = small.tile([P, 4], BF16, tag="vp")
            nc.gpsimd.tensor_scalar_mul(vp[:], vT[:, i, :], inv[:])
            pending.append((vp, E_sb))
            # numer matmuls run one block behind so the PE never starves the
            # ACT/DVE exp of the current block
            if len(pending) > 1:
                pvp, pE = pending.pop(0)
                ip = i - 1
                for j in range(n_mch):
                    jj, bk = j % 4, j // 4
                    nc.tensor.matmul(
                        numer_ps[bk][32 * jj:32 * jj + 4, :], pvp[:],
                        pE[:, j * 512:(j + 1) * 512],
                        start=(ip == 0), stop=False,
                        tile_position=(0, 32 * jj))

        # drain the last pending block's numer matmuls
        pvp, pE = pending.pop(0)
        for j in range(n_mch):
            jj, bk = j % 4, j // 4
            nc.tensor.matmul(
                numer_ps[bk][32 * jj:32 * jj + 4, :], pvp[:],
                pE[:, j * 512:(j + 1) * 512],
                start=(nb == 1), stop=True,
                tile_position=(0, 32 * jj))

        # ---- final: out = alpha * numer/(1e-9+colsum) + x ----
        epsb = consts.tile([P, 1], F32)
        nc.vector.memset(epsb[:], 1e-9)
        qs3 = (nc.gpsimd, nc.sync, nc.scalar)
        for bk in range(n_banks):
            # separate tiles per bank so each bank's chain has no false deps
            recip_b = sb.tile([P, 512], F32, tag=f"recip{bk}", name=f"recip{bk}")
            rep_b = sb.tile([P, 512], F32, tag=f"rep{bk}", name=f"rep{bk}")
            nc.vector.memset(rep_b[:], 0.0)
            nc.scalar.activation(out=recip_b[:], in_=numer_ps[bk][:],
                                 func=Ident, bias=epsb[:], scale=1.0)
            nc.vector.reciprocal(recip_b[:], recip_b[:])
            for jj in range(min(4, n_mch - 4 * bk)):
                srow = recip_b[32 * jj + 3:32 * jj + 4, :]
                # free-dim step-0 broadcast: re-read the same 512 row 4x
                # while the dst walks 4 partitions (partition step 0 is
                # not allowed on SBUF APs)
                src_b = bass.AP(tensor=srow.tensor, offset=srow.offset,
                                ap=[list(srow.ap[0]), [0, 4], list(srow.ap[-1])])
                qs3[jj % 3].dma_start(rep_b[32 * jj:32 * jj + 4, :], src_b)
            att_b = sb.tile([P, 512], F32, tag=f"att{bk}", name=f"att{bk}")
            nc.vector.tensor_mul(att_b[:], numer_ps[bk][:], rep_b[:])
            # out = alpha*att + x in the scattered numer layout (x4 is
            # host-prepared in the same layout), DMA'd straight to DRAM
            out_b = sb.tile([P, 512], F32, tag=f"osc{bk}", name=f"osc{bk}")
            nc.vector.scalar_tensor_tensor(
                out=out_b[:], in0=att_b[:], scalar=alphav[:],
                in1=x4_sb[:, bk, :], op0=AL.mult, op1=AL.add)
            for jj in range(min(4, n_mch - 4 * bk)):
                j = 4 * bk + jj
                qs3[(jj + 1) % 3].dma_start(
                    out_d.ap()[:, j * 512:(j + 1) * 512],
                    out_b[32 * jj:32 * jj + 3, :])

    nc.compile()
    return nc


def fold_weights(inputs):
    """Host-side BN folding. Returns the per-core constant input dict."""
    import ml_dtypes
    bf16 = ml_dtypes.bfloat16

    def fold(w, g, b, m, v):
        s = (g / np.sqrt(v + BN_EPS)).astype(np.float64)
        t = b.astype(np.float64) - s * m.astype(np.float64)
        return s[:, None] * w.astype(np.float64), t

    w1p, t1 = fold(inputs["w1"], inputs["g1"], inputs["b1"],
                   inputs["m1"], inputs["v1"])
    t1 = t1 + float(np.asarray(inputs["offset"]).ravel()[0]) * w1p.sum(axis=1)
    wqp, tq = fold(inputs["wq"], inputs["gq"], inputs["bq"],
                   inputs["mq"], inputs["vq"])
    wkp, tk = fold(inputs["wk"], inputs["gk"], inputs["bk"],
                   inputs["mk"], inputs["vk"])
    wvp, tv = fold(inputs["wv"], inputs["gv"], inputs["bv"],
                   inputs["mv"], inputs["vv"])
    w2 = np.asarray(inputs["w2"]).astype(np.float64)
    wqkv = np.concatenate([wqp, wkp, wvp], axis=0) @ w2   # [35, 64]
    tqkv = np.concatenate([tq, tk, tv], axis=0)           # [35]
    alpha = float(np.asarray(inputs["alpha"]).ravel()[0])
    return {
        "w1t": np.ascontiguousarray(w1p.T).astype(bf16),
        "t1": t1.astype(np.float32).reshape(64, 1),
        "wqkvt": np.ascontiguousarray(wqkv.T).astype(bf16),
        "tqkv": tqkv.astype(np.float32).reshape(35, 1),
        "alphav": np.full((128, 1), alpha, np.float32),
    }


_prog_cache = {}


def get_program(n=N, n_cores=N_CORES):
    key = (n, n_cores)
    if key not in _prog_cache:
        _prog_cache[key] = build_program(n, n_cores)
    return _prog_cache[key]


def make_x4(xb, n=N):
    """Scatter x [3, n] into the numer psum layout [128, n_banks, 512]."""
    n_mch = n // 512
    n_banks = (n_mch + 3) // 4
    x4 = np.zeros((128, n_banks, 512), np.float32)
    for j in range(n_mch):
        jj, bk = j % 4, j // 4
        x4[32 * jj:32 * jj + 3, bk, :] = xb[:, j * 512:(j + 1) * 512]
    return x4


def kernel(_trace=False, _trace_kwargs=None, **inputs):
    import ml_dtypes
    inputs = {k: np.asarray(v) for k, v in inputs.items()}
    nc = get_program()
    const_ins = fold_weights(inputs)
    x = inputs["x"].astype(np.float32)
    in_maps = [dict(const_ins,
                    x4=make_x4(x[b]),
                    xbf=np.ascontiguousarray(x[b]).astype(ml_dtypes.bfloat16))
               for b in range(B)]
    res = run_bass_kernel_spmd(nc, in_maps, core_ids=list(range(N_CORES)),
                               trace=_trace, **(_trace_kwargs or {}))
    out = np.stack([res.results[b]["out"] for b in range(B)], axis=0)
    if _trace:
        kernel.last_result = res
    return out.astype(np.float32)


if __name__ == "__main__":
    t0 = time.time()
    nc = get_program()
    print("build+compile:", time.time() - t0, flush=True)


# revision 32
# speedup vs baseline: 1.0118x; 1.0070x over previous
"""Point spatial attention (offset-attention) Trainium2 kernel.

Data-parallel over batch B=8 across 8 NeuronCores; each core runs one
point cloud (N=4096) end-to-end:

  feat = w2 @ relu(bn1(w1 @ (x+offset)))          [128, N]
  q/k/v = relu(bn(w @ feat))                      [16/16/3, N]
  energy = q^T k                                  [N, N]
  sim = softmax_row(energy); sim /= colsum(sim)
  out = alpha * (v @ sim) + x                     [3, N]

Device algorithm (single pass over the [N, N] matrix, ~135 us/core by
the instruction cost model):
  - BN affines folded into conv weights host-side; w2 is folded into
    the q/k/v weights too (no nonlinearity between them), so the head
    is just two small matmul stages.
  - All matmul operands in bf16 (fp32 moving operands stream at 1/4
    rate on the PE); accumulation stays fp32 in PSUM.  Energies are
    ~0.04 and the near-uniform softmax averages the bf16 rounding away
    (measured 3e-9 scale-relative final error vs the f32 reference).
  - Softmax without max-subtraction (energy in [0, 0.08]; exp of that
    range is exact-safe in f32).
  - Per 128-row block i: E_i = exp(q_i^T k), split between the ACT
    engine (ACTIVATE Exp, row-sum fused via accum_out) and the DVE (a
    custom fused op computing a quadratic fit of exp + accumulate in
    one pass) so both engines share the N^2 exp bottleneck.  Then
    v'_i = [v; 1]^T / rowsum and numer += v'_i^T E_i accumulates in
    PSUM across all 32 blocks, one block behind the exp pipeline so
    the PE never starves the exp engines.  The extra ones-row of v'
    yields colsum(sim), making the final column normalization a
    reciprocal+multiply at the end.
  - numer PSUM lives in 2 banks: 8 m-chunks of [4, 512] packed at
    partition offsets 0/32/64/96 via tensor-engine column tiling,
    leaving 6 banks for triple-buffered energy/exp chunks.
"""

import time
from contextlib import ExitStack

import numpy as np

import concourse.bass as bass
import concourse.mybir as mybir
import concourse.tile as tile
from concourse import bacc
from concourse.bass_utils import run_bass_kernel_spmd
from concourse.masks import make_identity


def _register_exp_poly():
    """Fused quadratic-poly exp with row-sum accumulate, one DVE pass:
    out = ((x + s0) * x) * s1 + imm2;  accum_out = sum(out).
    Registered at import into dve_ops.OPS (runtime append, row 17+)."""
    from operator import add as _add
    import concourse.dve_ops as dve_ops
    from concourse.dve_spec import Spec, Src0, C0, C1, C2, lower
    from concourse.dve_uop import DveOpSpec
    from concourse.dve_table_gen import dve_ver_for

    name = "EXP_POLY_ACC_ANT"
    if name in dve_ops._SUB_OPCODE_FOR_NAME:
        return next(op for op in dve_ops.OPS if op.name == name)

    def _ref(in0, in1, c0, c1, c2):
        b = (((in0.astype(np.float32) + c0) * in0) * c1 + c2).astype(np.float32)
        return b, b.reshape(b.shape[0], -1).sum(axis=-1, keepdims=True)

    spec = Spec(body=((Src0 + C0) * Src0) * C1 + C2, accum=_add, reference=_ref)
    row = dve_ops._CUSTOM_DVE_ROW_BASE + len(dve_ops.OPS)
    assert row < 0x20
    shas = {}
    for ver in ("v3", "v4"):
        ds = DveOpSpec(name=name, opcode=row, uops=lower(spec, ver=ver),
                       rd1_en=False)
        shas[ver] = ds.sha(ver)
    op = dve_ops.DveOp(name, spec, subdim=False, uops_sha=shas)
    dve_ops.OPS.append(op)
    dve_ops._SUB_OPCODE_FOR_NAME[name] = row
    dve_ops.CUSTOM_DVE_SPECS[name] = spec
    return op


EXP_POLY = _register_exp_poly()

F32 = mybir.dt.float32
BF16 = mybir.dt.bfloat16
BN_EPS = 1e-5
N = 4096
B = 8
N_CORES = 8
P = 128


def _chunks(total, maxc):
    out = []
    rem = total
    while rem > 0:
        c = min(maxc, rem)
        out.append((total - rem, c))
        rem -= c
    return out


def build_program(n=N, n_cores=N_CORES):
    nc = bacc.Bacc("TRN2", target_bir_lowering=False, debug=False,
                   num_devices=n_cores)
    nb = n // P           # row blocks
    n_mch = n // 512      # m-chunks for the numer matmuls (<= 8)
    n_banks = (n_mch + 3) // 4   # numer psum banks
    if n >= 4096:
        # (offset, len, engine): ACT does exp, DVE does the fused poly-exp
        ech = [(0, 1024, "A"), (1024, 1024, "A"),
               (2048, 1024, "D"), (3072, 1024, "D")]
    else:
        ech = [(off, ln, ("D" if len(_chunks(n, 1536)) >= 2
                          and i == len(_chunks(n, 1536)) - 1 else "A"))
               for i, (off, ln) in enumerate(_chunks(n, 1536))]
    assert n_mch <= 8 and n % 1024 == 0 and all(c[1] <= 1024 for c in ech)

    x4_d = nc.dram_tensor("x4", [P, n_banks, 512], F32, kind="ExternalInput")
    xbf_d = nc.dram_tensor("xbf", [3, n], BF16, kind="ExternalInput")
    w1t_d = nc.dram_tensor("w1t", [3, 64], BF16, kind="ExternalInput")
    t1_d = nc.dram_tensor("t1", [64, 1], F32, kind="ExternalInput")
    wqkvt_d = nc.dram_tensor("wqkvt", [64, 35], BF16, kind="ExternalInput")
    tqkv_d = nc.dram_tensor("tqkv", [35, 1], F32, kind="ExternalInput")
    alpha_d = nc.dram_tensor("alphav", [P, 1], F32, kind="ExternalInput")
    out_d = nc.dram_tensor("out", [3, n], F32, kind="ExternalOutput")

    AL = mybir.AluOpType
    Exp = mybir.ActivationFunctionType.Exp
    Relu = mybir.ActivationFunctionType.Relu
    Ident = mybir.ActivationFunctionType.Identity
    AX = mybir.AxisListType.X

    with ExitStack() as ctx:
        tc = ctx.enter_context(tile.TileContext(nc))
        consts = ctx.enter_context(tc.tile_pool(name="consts", bufs=1))
        sb = ctx.enter_context(tc.tile_pool(name="sb", bufs=1))
        epool = ctx.enter_context(tc.tile_pool(name="epsum", bufs=3, space="PSUM"))
        npool = ctx.enter_context(tc.tile_pool(name="npsum", bufs=1, space="PSUM"))
        Epool = ctx.enter_context(tc.tile_pool(name="Esb", bufs=3))
        small = ctx.enter_context(tc.tile_pool(name="small", bufs=4))

        # ---- constant loads (weights first; tail-only tensors later) ----
        w1t = consts.tile([3, 64], BF16)
        nc.sync.dma_start(w1t[:], w1t_d.ap()[:])
        t1 = consts.tile([64, 1], F32)
        nc.sync.dma_start(t1[:], t1_d.ap()[:])
        wqkvt = consts.tile([64, 35], BF16)
        nc.gpsimd.dma_start(wqkvt[:], wqkvt_d.ap()[:])
        tqkv = consts.tile([35, 1], F32)
        nc.gpsimd.dma_start(tqkv[:], tqkv_d.ap()[:])
        xbf_sb = consts.tile([3, n], BF16)
        for c in range(n // 1024):
            sl = slice(c * 1024, (c + 1) * 1024)
            nc.sync.dma_start(xbf_sb[:, sl], xbf_d.ap()[:, sl])
        x4_sb = consts.tile([P, n_banks, 512], F32)
        nc.gpsimd.dma_start(x4_sb[:], x4_d.ap()[:])
        alphav = consts.tile([P, 1], F32)
        nc.gpsimd.dma_start(alphav[:], alpha_d.ap()[:])

        # ---- head (chunk-interleaved):
        #   r1 = relu(w1' x + t1')  [DVE]
        #   qkv = relu((Wqkv w2) r1 + tqkv)  [ACT]; rows 0-15 q, 16-31 k, 32-34 v
        # (w2 is folded into the qkv weights host-side -- no feat stage)
        r1_sb = sb.tile([64, n], BF16)
        qkv_sb = sb.tile([35, n], BF16)
        k_sb = sb.tile([16, n], BF16)
        v_sb = sb.tile([3, n], BF16)
        ident = consts.tile([3, 3], BF16)
        make_identity(nc, ident)
        # h1 runs one chunk ahead of qp so the PE fills the r1 wait
        nch = n // 1024
        for c in range(nch + 1):
            if c < nch:
                h1 = epool.tile([P, 1024], F32, tag="e")
                for s in range(2):
                    sl = slice(c * 1024 + s * 512, c * 1024 + (s + 1) * 512)
                    nc.tensor.matmul(h1[0:64, s * 512:(s + 1) * 512],
                                     w1t[:], xbf_sb[:, sl], start=True, stop=True)
                for s in range(2):
                    nc.vector.tensor_scalar(
                        out=r1_sb[:, c * 1024 + s * 512:c * 1024 + (s + 1) * 512],
                        in0=h1[0:64, s * 512:(s + 1) * 512],
                        scalar1=t1[:], scalar2=0.0, op0=AL.add, op1=AL.max)
            if c > 0:
                cq = c - 1
                ch = slice(cq * 1024, (cq + 1) * 1024)
                qp = epool.tile([P, 1024], F32, tag="e")
                for s in range(2):
                    sl = slice(cq * 1024 + s * 512, cq * 1024 + (s + 1) * 512)
                    nc.tensor.matmul(qp[0:35, s * 512:(s + 1) * 512],
                                     wqkvt[:], r1_sb[:, sl], start=True, stop=True)
                nc.scalar.activation(
                    out=qkv_sb[:, ch], in_=qp[0:35, 0:1024],
                    func=Relu, bias=tqkv[:], scale=1.0)
                # k/v shifted to base partition 0 as each chunk lands
                nc.sync.dma_start(k_sb[:, ch], qkv_sb[16:32, ch])
                nc.gpsimd.dma_start(v_sb[:, ch], qkv_sb[32:35, ch])
        # v transposes (batched after the loop; v chunks landed during it)
        assert 4 * nb <= 2048
        tp = epool.tile([P, 2048], BF16, tag="e", name="tp")
        for i in range(nb):
            nc.tensor.transpose(tp[:, 4 * i:4 * i + 3],
                                v_sb[:, i * P:(i + 1) * P], ident[:])

        # vT_ext [128, nb, 4] bf16: cols 0-2 = v^T, col 3 = 1.0 (colsum
        # carrier); the per-chunk transposes above landed in tp
        vT = sb.tile([P, nb, 4], BF16)
        nc.vector.memset(vT[:], 1.0)
        tp4 = tp[:, 0:4 * nb].rearrange("p (a b) -> p a b", b=4)
        nc.vector.tensor_copy(vT[:, :, 0:3], tp4[:, :, 0:3])

        # numer accumulators: m-chunk j -> bank j//4, partitions 32*(j%4)+0..3
        numer_ps = []
        for bk in range(n_banks):
            nt = npool.tile([P, 512], F32, tag=f"numer{bk}", name=f"numer{bk}")
            nc.vector.memset(nt[:], 0.0)
            numer_ps.append(nt)

        # quadratic Chebyshev fit of exp on [0, 0.25] for the DVE-side exp
        # (energies are >= 0 since q,k are post-relu; observed max ~0.073,
        # fit error ~1e-5 -- far below the bf16 storage rounding of E):
        # exp(x) ~= c2*x^2 + c1*x + c0 = ((x + c1/c2) * x) * c2 + c0
        _xs = np.cos(np.pi * (np.arange(64) + 0.5) / 64) * 0.125 + 0.125
        _cf = np.polyfit(_xs, np.exp(_xs), 2)
        PC2, PC1, PC0 = float(_cf[0]), float(_cf[1]), float(_cf[2])

        # ---- main loop over row blocks ----
        pending = []
        for i in range(nb):
            E_sb = Epool.tile([P, n], BF16, tag="E")
            racc = small.tile([P, max(len(ech), 2)], F32, tag="racc")
            for ci, (off, ln, eng) in enumerate(ech):
                e_ps = epool.tile([P, 1024], F32, tag="e")
                for s in range(0, ln, 512):
                    sl = slice(off + s, off + s + 512)
                    nc.tensor.matmul(e_ps[:, s:s + 512],
                                     qkv_sb[0:16, i * P:(i + 1) * P],
                                     k_sb[:, sl], start=True, stop=True)
                if eng == "D":
                    nc.vector._custom_dve(
                        EXP_POLY, out=E_sb[:, off:off + ln],
                        in0=e_ps[:, 0:ln], s0=PC1 / PC2, s1=PC2, imm2=PC0,
                        accum_out=racc[:, ci:ci + 1])
                else:
                    nc.scalar.activation(
                        out=E_sb[:, off:off + ln], in_=e_ps[:, 0:ln],
                        func=Exp, accum_out=racc[:, ci:ci + 1])
            rs = small.tile([P, 1], F32, tag="rs")
            nc.vector.reduce_sum(rs[:], racc[:, 0:len(ech)], axis=AX)
            inv = small.tile([P, 1], F32, tag="inv")
            nc.vector.reciprocal(inv[:], rs[:])
            vp = small.tile([P, 4], BF16, tag="vp")
            nc.gpsimd.tensor_scalar_mul(vp[:], vT[:, i, :], inv[:])
            pending.append((vp, E_sb))
            # numer matmuls run one block behind so the PE never starves the
            # ACT/DVE exp of the current block
            if len(pending) > 1:
                pvp, pE = pending.pop(0)
                ip = i - 1
                for j in range(n_mch):
                    jj, bk = j % 4, j // 4
                    nc.tensor.matmul(
                        numer_ps[bk][32 * jj:32 * jj + 4, :], pvp[:],
                        pE[:, j * 512:(j + 1) * 512],
                        start=(ip == 0), stop=False,
                        tile_position=(0, 32 * jj))

        # drain the last pending block's numer matmuls
        pvp, pE = pending.pop(0)
        for j in range(n_mch):
            jj, bk = j % 4, j // 4
            nc.tensor.matmul(
                numer_ps[bk][32 * jj:32 * jj + 4, :], pvp[:],
                pE[:, j * 512:(j + 1) * 512],
                start=(nb == 1), stop=True,
                tile_position=(0, 32 * jj))

        # ---- final: out = alpha * numer/(1e-9+colsum) + x ----
        epsb = consts.tile([P, 1], F32)
        nc.vector.memset(epsb[:], 1e-9)
        qs3 = (nc.gpsimd, nc.sync, nc.scalar)
        for bk in range(n_banks):
            # separate tiles per bank so each bank's chain has no false deps
            recip_b = sb.tile([P, 512], F32, tag=f"recip{bk}", name=f"recip{bk}")
            rep_b = sb.tile([P, 512], F32, tag=f"rep{bk}", name=f"rep{bk}")
            nc.vector.memset(rep_b[:], 0.0)
            nc.scalar.activation(out=recip_b[:], in_=numer_ps[bk][:],
                                 func=Ident, bias=epsb[:], scale=1.0)
            nc.vector.reciprocal(recip_b[:], recip_b[:])
            for jj in range(min(4, n_mch - 4 * bk)):
                srow = recip_b[32 * jj + 3:32 * jj + 4, :]
                # free-dim step-0 broadcast: re-read the same 512 row 4x
                # while the dst walks 4 partitions (partition step 0 is
                # not allowed on SBUF APs)
                src_b = bass.AP(tensor=srow.tensor, offset=srow.offset,
                                ap=[list(srow.ap[0]), [0, 4], list(srow.ap[-1])])
                qs3[jj % 3].dma_start(rep_b[32 * jj:32 * jj + 4, :], src_b)
            att_b = sb.tile([P, 512], F32, tag=f"att{bk}", name=f"att{bk}")
            nc.vector.tensor_mul(att_b[:], numer_ps[bk][:], rep_b[:])
            # out = alpha*att + x in the scattered numer layout (x4 is
            # host-prepared in the same layout), DMA'd straight to DRAM
            out_b = sb.tile([P, 512], F32, tag=f"osc{bk}", name=f"osc{bk}")
            nc.vector.scalar_tensor_tensor(
                out=out_b[:], in0=att_b[:], scalar=alphav[:],
                in1=x4_sb[:, bk, :], op0=AL.mult, op1=AL.add)
            for jj in range(min(4, n_mch - 4 * bk)):
                j = 4 * bk + jj
                qs3[(jj + 1) % 3].dma_start(
                    out_d.ap()[:, j * 512:(j + 1) * 512],
                    out_b[32 * jj:32 * jj + 3, :])

    nc.compile()
    return nc


def fold_weights(inputs):
    """Host-side BN folding. Returns the per-core constant input dict."""
    import ml_dtypes
    bf16 = ml_dtypes.bfloat16

    def fold(w, g, b, m, v):
        s = (g / np.sqrt(v + BN_EPS)).astype(np.float64)
        t = b.astype(np.float64) - s * m.astype(np.float64)
        return s[:, None] * w.astype(np.float64), t

    w1p, t1 = fold(inputs["w1"], inputs["g1"], inputs["b1"],
                   inputs["m1"], inputs["v1"])
    t1 = t1 + float(np.asarray(inputs["offset"]).ravel()[0]) * w1p.sum(axis=1)
    wqp, tq = fold(inputs["wq"], inputs["gq"], inputs["bq"],
                   inputs["mq"], inputs["vq"])
    wkp, tk = fold(inputs["wk"], inputs["gk"], inputs["bk"],
                   inputs["mk"], inputs["vk"])
    wvp, tv = fold(inputs["wv"], inputs["gv"], inputs["bv"],
                   inputs["mv"], inputs["vv"])
    w2 = np.asarray(inputs["w2"]).astype(np.float64)
    wqkv = np.concatenate([wqp, wkp, wvp], axis=0) @ w2   # [35, 64]
    tqkv = np.concatenate([tq, tk, tv], axis=0)           # [35]
    alpha = float(np.asarray(inputs["alpha"]).ravel()[0])
    return {
        "w1t": np.ascontiguousarray(w1p.T).astype(bf16),
        "t1": t1.astype(np.float32).reshape(64, 1),
        "wqkvt": np.ascontiguousarray(wqkv.T).astype(bf16),
        "tqkv": tqkv.astype(np.float32).reshape(35, 1),
        "alphav": np.full((128, 1), alpha, np.float32),
    }


_prog_cache = {}


def get_program(n=N, n_cores=N_CORES):
    key = (n, n_cores)
    if key not in _prog_cache:
        _prog_cache[key] = build_program(n, n_cores)
    return _prog_cache[key]


def make_x4(xb, n=N):
    """Scatter x [3, n] into the numer psum layout [128, n_banks, 512]."""
    n_mch = n // 512
    n_banks = (n_mch + 3) // 4
    x4 = np.zeros((128, n_banks, 512), np.float32)
    for j in range(n_mch):
        jj, bk = j % 4, j // 4
        x4[32 * jj:32 * jj + 3, bk, :] = xb[:, j * 512:(j + 1) * 512]
    return x4


def kernel(_trace=False, _trace_kwargs=None, **inputs):
    import ml_dtypes
    inputs = {k: np.asarray(v) for k, v in inputs.items()}
    nc = get_program()
    const_ins = fold_weights(inputs)
    x = inputs["x"].astype(np.float32)
    in_maps = [dict(const_ins,
                    x4=make_x4(x[b]),
                    xbf=np.ascontiguousarray(x[b]).astype(ml_dtypes.bfloat16))
               for b in range(B)]
    res = run_bass_kernel_spmd(nc, in_maps, core_ids=list(range(N_CORES)),
                               trace=_trace, **(_trace_kwargs or {}))
    out = np.stack([res.results[b]["out"] for b in range(B)], axis=0)
    if _trace:
        kernel.last_result = res
    return out.astype(np.float32)


if __name__ == "__main__":
    t0 = time.time()
    nc = get_program()
    print("build+compile:", time.time() - t0, flush=True)


# revision 35
# speedup vs baseline: 1.0480x; 1.0358x over previous
"""Point spatial attention (offset-attention) Trainium2 kernel.

Data-parallel over batch B=8 across 8 NeuronCores; each core runs one
point cloud (N=4096) end-to-end:

  feat = w2 @ relu(bn1(w1 @ (x+offset)))          [128, N]
  q/k/v = relu(bn(w @ feat))                      [16/16/3, N]
  energy = q^T k                                  [N, N]
  sim = softmax_row(energy); sim /= colsum(sim)
  out = alpha * (v @ sim) + x                     [3, N]

Device algorithm (single pass over the [N, N] matrix, ~135 us/core by
the instruction cost model):
  - BN affines folded into conv weights host-side; w2 is folded into
    the q/k/v weights too (no nonlinearity between them), so the head
    is just two small matmul stages.
  - All matmul operands in bf16 (fp32 moving operands stream at 1/4
    rate on the PE); accumulation stays fp32 in PSUM.  Energies are
    ~0.04 and the near-uniform softmax averages the bf16 rounding away
    (measured 3e-9 scale-relative final error vs the f32 reference).
  - Softmax without max-subtraction (energy in [0, 0.08]; exp of that
    range is exact-safe in f32).
  - Per 128-row block i: E_i = exp(q_i^T k), split between the ACT
    engine (ACTIVATE Exp, row-sum fused via accum_out) and the DVE (a
    custom fused op computing a quadratic fit of exp + accumulate in
    one pass) so both engines share the N^2 exp bottleneck.  Then
    v'_i = [v; 1]^T / rowsum and numer += v'_i^T E_i accumulates in
    PSUM across all 32 blocks, one block behind the exp pipeline so
    the PE never starves the exp engines.  The extra ones-row of v'
    yields colsum(sim), making the final column normalization a
    reciprocal+multiply at the end.
  - numer PSUM lives in 2 banks: 8 m-chunks of [4, 512] packed at
    partition offsets 0/32/64/96 via tensor-engine column tiling,
    leaving 6 banks for triple-buffered energy/exp chunks.
"""

import time
from contextlib import ExitStack

import numpy as np

import concourse.bass as bass
import concourse.mybir as mybir
import concourse.tile as tile
from concourse import bacc
from concourse.bass_utils import run_bass_kernel_spmd
from concourse.masks import make_identity


def _register_exp_poly():
    """Fused quadratic-poly exp with row-sum accumulate, one DVE pass:
    out = ((x + s0) * x) * s1 + imm2;  accum_out = sum(out).
    Registered at import into dve_ops.OPS (runtime append, row 17+)."""
    from operator import add as _add
    import concourse.dve_ops as dve_ops
    from concourse.dve_spec import Spec, Src0, C0, C1, C2, lower
    from concourse.dve_uop import DveOpSpec
    from concourse.dve_table_gen import dve_ver_for

    name = "EXP_POLY_ACC_ANT"
    if name in dve_ops._SUB_OPCODE_FOR_NAME:
        return next(op for op in dve_ops.OPS if op.name == name)

    def _ref(in0, in1, c0, c1, c2):
        b = (((in0.astype(np.float32) + c0) * in0) * c1 + c2).astype(np.float32)
        return b, b.reshape(b.shape[0], -1).sum(axis=-1, keepdims=True)

    spec = Spec(body=((Src0 + C0) * Src0) * C1 + C2, accum=_add, reference=_ref)
    row = dve_ops._CUSTOM_DVE_ROW_BASE + len(dve_ops.OPS)
    assert row < 0x20
    shas = {}
    for ver in ("v3", "v4"):
        ds = DveOpSpec(name=name, opcode=row, uops=lower(spec, ver=ver),
                       rd1_en=False)
        shas[ver] = ds.sha(ver)
    op = dve_ops.DveOp(name, spec, subdim=False, uops_sha=shas)
    dve_ops.OPS.append(op)
    dve_ops._SUB_OPCODE_FOR_NAME[name] = row
    dve_ops.CUSTOM_DVE_SPECS[name] = spec
    return op


EXP_POLY = _register_exp_poly()

F32 = mybir.dt.float32
BF16 = mybir.dt.bfloat16
FP8 = mybir.dt.float8e4
BN_EPS = 1e-5
N = 4096
B = 8
N_CORES = 8
P = 128


def _chunks(total, maxc):
    out = []
    rem = total
    while rem > 0:
        c = min(maxc, rem)
        out.append((total - rem, c))
        rem -= c
    return out


def build_program(n=N, n_cores=N_CORES):
    nc = bacc.Bacc("TRN2", target_bir_lowering=False, debug=False,
                   num_devices=n_cores)
    nb = n // P           # row blocks
    n_mch = n // 512      # m-chunks for the numer matmuls (<= 8)
    n_banks = (n_mch + 3) // 4   # numer psum banks
    if n >= 4096:
        # (offset, len, engine): ACT does exp, DVE does the fused poly-exp
        ech = [(0, 1024, "A"), (1024, 1024, "A"),
               (2048, 1024, "D"), (3072, 1024, "D")]
    else:
        ech = [(off, ln, ("D" if len(_chunks(n, 1024)) >= 2
                          and i == len(_chunks(n, 1024)) - 1 else "A"))
               for i, (off, ln) in enumerate(_chunks(n, 1024))]
    assert n_mch <= 8 and n % 1024 == 0 and all(c[1] <= 1024 for c in ech)

    x4_d = nc.dram_tensor("x4", [P, n_banks, 512], F32, kind="ExternalInput")
    xbf_d = nc.dram_tensor("xbf", [3, n], BF16, kind="ExternalInput")
    w1t_d = nc.dram_tensor("w1t", [3, 64], BF16, kind="ExternalInput")
    t1_d = nc.dram_tensor("t1", [64, 1], F32, kind="ExternalInput")
    wqkvt_d = nc.dram_tensor("wqkvt", [64, 35], BF16, kind="ExternalInput")
    tqkv_d = nc.dram_tensor("tqkv", [35, 1], F32, kind="ExternalInput")
    alpha_d = nc.dram_tensor("alphav", [P, 1], F32, kind="ExternalInput")
    out_d = nc.dram_tensor("out", [3, n], F32, kind="ExternalOutput")

    AL = mybir.AluOpType
    Exp = mybir.ActivationFunctionType.Exp
    Relu = mybir.ActivationFunctionType.Relu
    Ident = mybir.ActivationFunctionType.Identity
    AX = mybir.AxisListType.X

    with ExitStack() as ctx:
        tc = ctx.enter_context(tile.TileContext(nc))
        consts = ctx.enter_context(tc.tile_pool(name="consts", bufs=1))
        sb = ctx.enter_context(tc.tile_pool(name="sb", bufs=1))
        epool = ctx.enter_context(tc.tile_pool(name="epsum", bufs=3, space="PSUM"))
        npool = ctx.enter_context(tc.tile_pool(name="npsum", bufs=1, space="PSUM"))
        Epool = ctx.enter_context(tc.tile_pool(name="Esb", bufs=3))
        small = ctx.enter_context(tc.tile_pool(name="small", bufs=4))
        dpool = ctx.enter_context(tc.tile_pool(name="dram", bufs=1, space="DRAM"))

        # ---- constant loads (weights first; tail-only tensors later) ----
        w1t = consts.tile([3, 64], BF16)
        nc.sync.dma_start(w1t[:], w1t_d.ap()[:])
        t1 = consts.tile([64, 1], F32)
        nc.sync.dma_start(t1[:], t1_d.ap()[:])
        wqkvt = consts.tile([64, 35], BF16)
        nc.gpsimd.dma_start(wqkvt[:], wqkvt_d.ap()[:])
        tqkv = consts.tile([35, 1], F32)
        nc.gpsimd.dma_start(tqkv[:], tqkv_d.ap()[:])
        xbf_sb = consts.tile([3, n], BF16)
        for c in range(n // 1024):
            sl = slice(c * 1024, (c + 1) * 1024)
            nc.sync.dma_start(xbf_sb[:, sl], xbf_d.ap()[:, sl])
        x4_sb = consts.tile([P, n_banks, 512], F32)
        nc.gpsimd.dma_start(x4_sb[:], x4_d.ap()[:])
        alphav = consts.tile([P, 1], F32)
        nc.gpsimd.dma_start(alphav[:], alpha_d.ap()[:])

        # ---- head (chunk-interleaved):
        #   r1 = relu(w1' x + t1')  [DVE]
        #   qkv = relu((Wqkv w2) r1 + tqkv)  [ACT]; rows 0-15 q, 16-31 k, 32-34 v
        # (w2 is folded into the qkv weights host-side -- no feat stage)
        r1_sb = sb.tile([64, n], BF16)
        qkv_sb = sb.tile([35, n], FP8)
        qk_d = dpool.tile([32, n], FP8)
        q_dr = sb.tile([8, 2, n], FP8)
        k_dr = sb.tile([8, 2, n], FP8)
        v_sb = sb.tile([3, n], BF16)
        ident = consts.tile([3, 3], BF16)
        make_identity(nc, ident)
        # h1 runs one chunk ahead of qp so the PE fills the r1 wait
        nch = n // 1024
        for c in range(nch + 1):
            if c < nch:
                h1 = epool.tile([P, 1024], F32, tag="e")
                for s in range(2):
                    sl = slice(c * 1024 + s * 512, c * 1024 + (s + 1) * 512)
                    nc.tensor.matmul(h1[0:64, s * 512:(s + 1) * 512],
                                     w1t[:], xbf_sb[:, sl], start=True, stop=True)
                for s in range(2):
                    nc.vector.tensor_scalar(
                        out=r1_sb[:, c * 1024 + s * 512:c * 1024 + (s + 1) * 512],
                        in0=h1[0:64, s * 512:(s + 1) * 512],
                        scalar1=t1[:], scalar2=0.0, op0=AL.add, op1=AL.max)
            if c > 0:
                cq = c - 1
                ch = slice(cq * 1024, (cq + 1) * 1024)
                qp = epool.tile([P, 1024], F32, tag="e")
                for s in range(2):
                    sl = slice(cq * 1024 + s * 512, cq * 1024 + (s + 1) * 512)
                    nc.tensor.matmul(qp[0:35, s * 512:(s + 1) * 512],
                                     wqkvt[:], r1_sb[:, sl], start=True, stop=True)
                nc.scalar.activation(
                    out=qkv_sb[:, ch], in_=qp[0:35, 0:1024],
                    func=Relu, bias=tqkv[:], scale=1.0)
                # q/k bounce through DRAM into the DoubleRow pair-
                # interleaved layout [8, 2, n] (channels 2p, 2p+1 share a
                # partition); v shifted to base partition 0
                nc.sync.dma_start(qk_d[:, ch], qkv_sb[0:32, ch])
                nc.sync.dma_start(
                    q_dr[:, :, ch],
                    qk_d[0:16, ch].rearrange("(p j) m -> p j m", j=2))
                nc.scalar.dma_start(
                    k_dr[:, :, ch],
                    qk_d[16:32, ch].rearrange("(p j) m -> p j m", j=2))
                # v in bf16 (fp8 PE transpose needs step-2 outputs)
                nc.vector.tensor_scalar(
                    out=v_sb[:, ch], in0=qp[32:35, 0:1024],
                    scalar1=tqkv[32:35, :], scalar2=0.0,
                    op0=AL.add, op1=AL.max)
        # v transposes (batched after the loop; v chunks landed during it)
        assert 4 * nb <= 2048
        tp = epool.tile([P, 2048], BF16, tag="e", name="tp")
        for i in range(nb):
            nc.tensor.transpose(tp[:, 4 * i:4 * i + 3],
                                v_sb[:, i * P:(i + 1) * P], ident[:])

        # vT_ext [128, nb, 4] bf16: cols 0-2 = v^T, col 3 = 1.0 (colsum
        # carrier); the per-chunk transposes above landed in tp
        vT = sb.tile([P, nb, 4], BF16)
        nc.vector.memset(vT[:], 1.0)
        tp4 = tp[:, 0:4 * nb].rearrange("p (a b) -> p a b", b=4)
        nc.vector.tensor_copy(vT[:, :, 0:3], tp4[:, :, 0:3])

        # numer accumulators: m-chunk j -> bank j//4, partitions 32*(j%4)+0..3
        numer_ps = []
        for bk in range(n_banks):
            nt = npool.tile([P, 512], F32, tag=f"numer{bk}", name=f"numer{bk}")
            nc.vector.memset(nt[:], 0.0)
            numer_ps.append(nt)

        # quadratic Chebyshev fit of exp on [0, 0.25] for the DVE-side exp
        # (energies are >= 0 since q,k are post-relu; observed max ~0.073,
        # fit error ~1e-5 -- far below the bf16 storage rounding of E):
        # exp(x) ~= c2*x^2 + c1*x + c0 = ((x + c1/c2) * x) * c2 + c0
        _xs = np.cos(np.pi * (np.arange(64) + 0.5) / 64) * 0.125 + 0.125
        _cf = np.polyfit(_xs, np.exp(_xs), 2)
        PC2, PC1, PC0 = float(_cf[0]), float(_cf[1]), float(_cf[2])

        # ---- main loop over row blocks ----
        pending = []
        for i in range(nb):
            E_sb = Epool.tile([P, n], BF16, tag="E")
            racc = small.tile([P, max(len(ech), 2)], F32, tag="racc")
            for ci, (off, ln, eng) in enumerate(ech):
                e_ps = epool.tile([P, 1024], F32, tag="e")
                for s in range(0, ln, 512):
                    sl = slice(off + s, off + s + 512)
                    nc.tensor.matmul(e_ps[:, s:s + 512],
                                     q_dr[:, :, i * P:(i + 1) * P],
                                     k_dr[:, :, sl], start=True, stop=True,
                                     perf_mode=mybir.MatmulPerfMode.DoubleRow)
                if eng == "D":
                    nc.vector._custom_dve(
                        EXP_POLY, out=E_sb[:, off:off + ln],
                        in0=e_ps[:, 0:ln], s0=PC1 / PC2, s1=PC2, imm2=PC0,
                        accum_out=racc[:, ci:ci + 1])
                else:
                    nc.scalar.activation(
                        out=E_sb[:, off:off + ln], in_=e_ps[:, 0:ln],
                        func=Exp, accum_out=racc[:, ci:ci + 1])
            rs = small.tile([P, 1], F32, tag="rs")
            nc.vector.reduce_sum(rs[:], racc[:, 0:len(ech)], axis=AX)
            inv = small.tile([P, 1], F32, tag="inv")
            nc.vector.reciprocal(inv[:], rs[:])
            vp = small.tile([P, 4], BF16, tag="vp")
            nc.gpsimd.tensor_scalar_mul(vp[:], vT[:, i, :], inv[:])
            pending.append((vp, E_sb))
            # numer matmuls run one block behind so the PE never starves the
            # ACT/DVE exp of the current block
            if len(pending) > 1:
                pvp, pE = pending.pop(0)
                ip = i - 1
                for j in range(n_mch):
                    jj, bk = j % 4, j // 4
                    nc.tensor.matmul(
                        numer_ps[bk][32 * jj:32 * jj + 4, :], pvp[:],
                        pE[:, j * 512:(j + 1) * 512],
                        start=(ip == 0), stop=False,
                        tile_position=(0, 32 * jj))

        # drain the last pending block's numer matmuls
        pvp, pE = pending.pop(0)
        for j in range(n_mch):
            jj, bk = j % 4, j // 4
            nc.tensor.matmul(
                numer_ps[bk][32 * jj:32 * jj + 4, :], pvp[:],
                pE[:, j * 512:(j + 1) * 512],
                start=(nb == 1), stop=True,
                tile_position=(0, 32 * jj))

        # ---- final: out = alpha * numer/(1e-9+colsum) + x ----
        epsb = consts.tile([P, 1], F32)
        nc.vector.memset(epsb[:], 1e-9)
        qs3 = (nc.gpsimd, nc.sync, nc.scalar)
        for bk in range(n_banks):
            # separate tiles per bank so each bank's chain has no false deps
            recip_b = sb.tile([P, 512], F32, tag=f"recip{bk}", name=f"recip{bk}")
            rep_b = sb.tile([P, 512], F32, tag=f"rep{bk}", name=f"rep{bk}")
            nc.vector.memset(rep_b[:], 0.0)
            nc.scalar.activation(out=recip_b[:], in_=numer_ps[bk][:],
                                 func=Ident, bias=epsb[:], scale=1.0)
            nc.vector.reciprocal(recip_b[:], recip_b[:])
            for jj in range(min(4, n_mch - 4 * bk)):
                srow = recip_b[32 * jj + 3:32 * jj + 4, :]
                # free-dim step-0 broadcast: re-read the same 512 row 4x
                # while the dst walks 4 partitions (partition step 0 is
                # not allowed on SBUF APs)
                src_b = bass.AP(tensor=srow.tensor, offset=srow.offset,
                                ap=[list(srow.ap[0]), [0, 4], list(srow.ap[-1])])
                qs3[jj % 3].dma_start(rep_b[32 * jj:32 * jj + 4, :], src_b)
            att_b = sb.tile([P, 512], F32, tag=f"att{bk}", name=f"att{bk}")
            nc.vector.tensor_mul(att_b[:], numer_ps[bk][:], rep_b[:])
            # out = alpha*att + x in the scattered numer layout (x4 is
            # host-prepared in the same layout), DMA'd straight to DRAM
            out_b = sb.tile([P, 512], F32, tag=f"osc{bk}", name=f"osc{bk}")
            nc.vector.scalar_tensor_tensor(
                out=out_b[:], in0=att_b[:], scalar=alphav[:],
                in1=x4_sb[:, bk, :], op0=AL.mult, op1=AL.add)
            for jj in range(min(4, n_mch - 4 * bk)):
                j = 4 * bk + jj
                qs3[(jj + 1) % 3].dma_start(
                    out_d.ap()[:, j * 512:(j + 1) * 512],
                    out_b[32 * jj:32 * jj + 3, :])

    nc.compile()
    return nc


def fold_weights(inputs):
    """Host-side BN folding. Returns the per-core constant input dict."""
    import ml_dtypes
    bf16 = ml_dtypes.bfloat16

    def fold(w, g, b, m, v):
        s = (g / np.sqrt(v + BN_EPS)).astype(np.float64)
        t = b.astype(np.float64) - s * m.astype(np.float64)
        return s[:, None] * w.astype(np.float64), t

    w1p, t1 = fold(inputs["w1"], inputs["g1"], inputs["b1"],
                   inputs["m1"], inputs["v1"])
    t1 = t1 + float(np.asarray(inputs["offset"]).ravel()[0]) * w1p.sum(axis=1)
    wqp, tq = fold(inputs["wq"], inputs["gq"], inputs["bq"],
                   inputs["mq"], inputs["vq"])
    wkp, tk = fold(inputs["wk"], inputs["gk"], inputs["bk"],
                   inputs["mk"], inputs["vk"])
    wvp, tv = fold(inputs["wv"], inputs["gv"], inputs["bv"],
                   inputs["mv"], inputs["vv"])
    w2 = np.asarray(inputs["w2"]).astype(np.float64)
    wqkv = np.concatenate([wqp, wkp, wvp], axis=0) @ w2   # [35, 64]
    tqkv = np.concatenate([tq, tk, tv], axis=0)           # [35]
    alpha = float(np.asarray(inputs["alpha"]).ravel()[0])
    return {
        "w1t": np.ascontiguousarray(w1p.T).astype(bf16),
        "t1": t1.astype(np.float32).reshape(64, 1),
        "wqkvt": np.ascontiguousarray(wqkv.T).astype(bf16),
        "tqkv": tqkv.astype(np.float32).reshape(35, 1),
        "alphav": np.full((128, 1), alpha, np.float32),
    }


_prog_cache = {}


def get_program(n=N, n_cores=N_CORES):
    key = (n, n_cores)
    if key not in _prog_cache:
        _prog_cache[key] = build_program(n, n_cores)
    return _prog_cache[key]


def make_x4(xb, n=N):
    """Scatter x [3, n] into the numer psum layout [128, n_banks, 512]."""
    n_mch = n // 512
    n_banks = (n_mch + 3) // 4
    x4 = np.zeros((128, n_banks, 512), np.float32)
    for j in range(n_mch):
        jj, bk = j % 4, j // 4
        x4[32 * jj:32 * jj + 3, bk, :] = xb[:, j * 512:(j + 1) * 512]
    return x4


def kernel(_trace=False, _trace_kwargs=None, **inputs):
    import ml_dtypes
    inputs = {k: np.asarray(v) for k, v in inputs.items()}
    nc = get_program()
    const_ins = fold_weights(inputs)
    x = inputs["x"].astype(np.float32)
    in_maps = [dict(const_ins,
                    x4=make_x4(x[b]),
                    xbf=np.ascontiguousarray(x[b]).astype(ml_dtypes.bfloat16))
               for b in range(B)]
    res = run_bass_kernel_spmd(nc, in_maps, core_ids=list(range(N_CORES)),
                               trace=_trace, **(_trace_kwargs or {}))
    out = np.stack([res.results[b]["out"] for b in range(B)], axis=0)
    if _trace:
        kernel.last_result = res
    return out.astype(np.float32)


if __name__ == "__main__":
    t0 = time.time()
    nc = get_program()
    print("build+compile:", time.time() - t0, flush=True)


# revision 36
# speedup vs baseline: 1.0718x; 1.0226x over previous
"""Point spatial attention (offset-attention) Trainium2 kernel.

Data-parallel over batch B=8 across 8 NeuronCores; each core runs one
point cloud (N=4096) end-to-end:

  feat = w2 @ relu(bn1(w1 @ (x+offset)))          [128, N]
  q/k/v = relu(bn(w @ feat))                      [16/16/3, N]
  energy = q^T k                                  [N, N]
  sim = softmax_row(energy); sim /= colsum(sim)
  out = alpha * (v @ sim) + x                     [3, N]

Device algorithm (single pass over the [N, N] matrix, ~135 us/core by
the instruction cost model):
  - BN affines folded into conv weights host-side; w2 is folded into
    the q/k/v weights too (no nonlinearity between them), so the head
    is just two small matmul stages.
  - All matmul operands in bf16 (fp32 moving operands stream at 1/4
    rate on the PE); accumulation stays fp32 in PSUM.  Energies are
    ~0.04 and the near-uniform softmax averages the bf16 rounding away
    (measured 3e-9 scale-relative final error vs the f32 reference).
  - Softmax without max-subtraction (energy in [0, 0.08]; exp of that
    range is exact-safe in f32).
  - Per 128-row block i: E_i = exp(q_i^T k), split between the ACT
    engine (ACTIVATE Exp, row-sum fused via accum_out) and the DVE (a
    custom fused op computing a quadratic fit of exp + accumulate in
    one pass) so both engines share the N^2 exp bottleneck.  Then
    v'_i = [v; 1]^T / rowsum and numer += v'_i^T E_i accumulates in
    PSUM across all 32 blocks, one block behind the exp pipeline so
    the PE never starves the exp engines.  The extra ones-row of v'
    yields colsum(sim), making the final column normalization a
    reciprocal+multiply at the end.
  - numer PSUM lives in 2 banks: 8 m-chunks of [4, 512] packed at
    partition offsets 0/32/64/96 via tensor-engine column tiling,
    leaving 6 banks for triple-buffered energy/exp chunks.
"""

import time
from contextlib import ExitStack

import numpy as np

import concourse.bass as bass
import concourse.mybir as mybir
import concourse.tile as tile
from concourse import bacc
from concourse.bass_utils import run_bass_kernel_spmd
from concourse.masks import make_identity


def _register_exp_poly():
    """Fused quadratic-poly exp with row-sum accumulate, one DVE pass:
    out = ((x + s0) * x) * s1 + imm2;  accum_out = sum(out).
    Registered at import into dve_ops.OPS (runtime append, row 17+)."""
    from operator import add as _add
    import concourse.dve_ops as dve_ops
    from concourse.dve_spec import Spec, Src0, C0, C1, C2, lower
    from concourse.dve_uop import DveOpSpec
    from concourse.dve_table_gen import dve_ver_for

    name = "EXP_POLY_ACC_ANT"
    if name in dve_ops._SUB_OPCODE_FOR_NAME:
        return next(op for op in dve_ops.OPS if op.name == name)

    def _ref(in0, in1, c0, c1, c2):
        b = (((in0.astype(np.float32) + c0) * in0) * c1 + c2).astype(np.float32)
        return b, b.reshape(b.shape[0], -1).sum(axis=-1, keepdims=True)

    spec = Spec(body=((Src0 + C0) * Src0) * C1 + C2, accum=_add, reference=_ref)
    row = dve_ops._CUSTOM_DVE_ROW_BASE + len(dve_ops.OPS)
    assert row < 0x20
    shas = {}
    for ver in ("v3", "v4"):
        ds = DveOpSpec(name=name, opcode=row, uops=lower(spec, ver=ver),
                       rd1_en=False)
        shas[ver] = ds.sha(ver)
    op = dve_ops.DveOp(name, spec, subdim=False, uops_sha=shas)
    dve_ops.OPS.append(op)
    dve_ops._SUB_OPCODE_FOR_NAME[name] = row
    dve_ops.CUSTOM_DVE_SPECS[name] = spec
    return op


EXP_POLY = _register_exp_poly()

F32 = mybir.dt.float32
BF16 = mybir.dt.bfloat16
FP8 = mybir.dt.float8e4
BN_EPS = 1e-5
N = 4096
B = 8
N_CORES = 8
P = 128


def _chunks(total, maxc):
    out = []
    rem = total
    while rem > 0:
        c = min(maxc, rem)
        out.append((total - rem, c))
        rem -= c
    return out


def build_program(n=N, n_cores=N_CORES):
    nc = bacc.Bacc("TRN2", target_bir_lowering=False, debug=False,
                   num_devices=n_cores)
    nb = n // P           # row blocks
    n_mch = n // 512      # m-chunks for the numer matmuls (<= 8)
    n_banks = (n_mch + 3) // 4   # numer psum banks
    if n >= 4096:
        # (offset, len, engine): ACT does exp, DVE does the fused poly-exp
        ech = [(0, 1024, "A"), (1024, 1024, "A"),
               (2048, 1024, "D"), (3072, 1024, "D")]
    else:
        ech = [(off, ln, ("D" if len(_chunks(n, 1024)) >= 2
                          and i == len(_chunks(n, 1024)) - 1 else "A"))
               for i, (off, ln) in enumerate(_chunks(n, 1024))]
    assert n_mch <= 8 and n % 1024 == 0 and all(c[1] <= 1024 for c in ech)

    x4_d = nc.dram_tensor("x4", [P, n_banks, 512], F32, kind="ExternalInput")
    xbf_d = nc.dram_tensor("xbf", [3, n], BF16, kind="ExternalInput")
    w1t_d = nc.dram_tensor("w1t", [3, 64], BF16, kind="ExternalInput")
    t1_d = nc.dram_tensor("t1", [64, 1], F32, kind="ExternalInput")
    wqkvt_d = nc.dram_tensor("wqkvt", [64, 35], BF16, kind="ExternalInput")
    tqkv_d = nc.dram_tensor("tqkv", [35, 1], F32, kind="ExternalInput")
    alpha_d = nc.dram_tensor("alphav", [P, 1], F32, kind="ExternalInput")
    out_d = nc.dram_tensor("out", [3, n], F32, kind="ExternalOutput")

    AL = mybir.AluOpType
    Exp = mybir.ActivationFunctionType.Exp
    Relu = mybir.ActivationFunctionType.Relu
    Ident = mybir.ActivationFunctionType.Identity
    AX = mybir.AxisListType.X

    with ExitStack() as ctx:
        tc = ctx.enter_context(tile.TileContext(nc))
        consts = ctx.enter_context(tc.tile_pool(name="consts", bufs=1))
        sb = ctx.enter_context(tc.tile_pool(name="sb", bufs=1))
        epool = ctx.enter_context(tc.tile_pool(name="epsum", bufs=3, space="PSUM"))
        npool = ctx.enter_context(tc.tile_pool(name="npsum", bufs=1, space="PSUM"))
        Epool = ctx.enter_context(tc.tile_pool(name="Esb", bufs=3))
        small = ctx.enter_context(tc.tile_pool(name="small", bufs=4))
        dpool = ctx.enter_context(tc.tile_pool(name="dram", bufs=1, space="DRAM"))

        # ---- constant loads (weights first; tail-only tensors later) ----
        w1t = consts.tile([3, 64], BF16)
        nc.sync.dma_start(w1t[:], w1t_d.ap()[:])
        t1 = consts.tile([64, 1], F32)
        nc.sync.dma_start(t1[:], t1_d.ap()[:])
        wqkvt = consts.tile([64, 35], BF16)
        nc.gpsimd.dma_start(wqkvt[:], wqkvt_d.ap()[:])
        tqkv = consts.tile([35, 1], F32)
        nc.gpsimd.dma_start(tqkv[:], tqkv_d.ap()[:])
        xbf_sb = consts.tile([3, n], BF16)
        for c in range(n // 1024):
            sl = slice(c * 1024, (c + 1) * 1024)
            nc.sync.dma_start(xbf_sb[:, sl], xbf_d.ap()[:, sl])
        x4_sb = consts.tile([P, n_banks, 512], F32)
        nc.gpsimd.dma_start(x4_sb[:], x4_d.ap()[:])
        alphav = consts.tile([P, 1], F32)
        nc.gpsimd.dma_start(alphav[:], alpha_d.ap()[:])

        # ---- head (chunk-interleaved):
        #   r1 = relu(w1' x + t1')  [DVE]
        #   qkv = relu((Wqkv w2) r1 + tqkv)  [ACT]; rows 0-15 q, 16-31 k, 32-34 v
        # (w2 is folded into the qkv weights host-side -- no feat stage)
        r1_sb = sb.tile([64, n], BF16)
        qkv_sb = sb.tile([35, n], FP8)
        qk_d = dpool.tile([32, n], FP8)
        q_dr = sb.tile([8, 2, n], FP8)
        k_dr = sb.tile([8, 2, n], FP8)
        v_sb = sb.tile([3, n], BF16)
        ident = consts.tile([3, 3], BF16)
        make_identity(nc, ident)
        # h1 runs one chunk ahead of qp so the PE fills the r1 wait
        nch = n // 1024
        for c in range(nch + 1):
            if c < nch:
                h1 = epool.tile([P, 1024], F32, tag="e")
                for s in range(2):
                    sl = slice(c * 1024 + s * 512, c * 1024 + (s + 1) * 512)
                    nc.tensor.matmul(h1[0:64, s * 512:(s + 1) * 512],
                                     w1t[:], xbf_sb[:, sl], start=True, stop=True)
                for s in range(2):
                    nc.vector.tensor_scalar(
                        out=r1_sb[:, c * 1024 + s * 512:c * 1024 + (s + 1) * 512],
                        in0=h1[0:64, s * 512:(s + 1) * 512],
                        scalar1=t1[:], scalar2=0.0, op0=AL.add, op1=AL.max)
            if c > 0:
                cq = c - 1
                ch = slice(cq * 1024, (cq + 1) * 1024)
                qp = epool.tile([P, 1024], F32, tag="e")
                for s in range(2):
                    sl = slice(cq * 1024 + s * 512, cq * 1024 + (s + 1) * 512)
                    nc.tensor.matmul(qp[0:35, s * 512:(s + 1) * 512],
                                     wqkvt[:], r1_sb[:, sl], start=True, stop=True)
                nc.scalar.activation(
                    out=qkv_sb[:, ch], in_=qp[0:35, 0:1024],
                    func=Relu, bias=tqkv[:], scale=1.0)
                # q/k bounce through DRAM into the DoubleRow pair-
                # interleaved layout [8, 2, n] (channels 2p, 2p+1 share a
                # partition); v shifted to base partition 0
                nc.sync.dma_start(qk_d[:, ch], qkv_sb[0:32, ch])
                nc.sync.dma_start(
                    q_dr[:, :, ch],
                    qk_d[0:16, ch].rearrange("(p j) m -> p j m", j=2))
                nc.scalar.dma_start(
                    k_dr[:, :, ch],
                    qk_d[16:32, ch].rearrange("(p j) m -> p j m", j=2))
                # v in bf16 (fp8 PE transpose needs step-2 outputs);
                # cast-copy from the relu'd fp8 qkv on the idle gpsimd
                nc.gpsimd.tensor_copy(v_sb[:, ch], qkv_sb[32:35, ch])
        # v transposes (batched after the loop; v chunks landed during it)
        assert 4 * nb <= 2048
        tp = epool.tile([P, 2048], BF16, tag="e", name="tp")
        for i in range(nb):
            nc.tensor.transpose(tp[:, 4 * i:4 * i + 3],
                                v_sb[:, i * P:(i + 1) * P], ident[:])

        # vT_ext [128, nb, 4] bf16: cols 0-2 = v^T, col 3 = 1.0 (colsum
        # carrier); the per-chunk transposes above landed in tp
        vT = sb.tile([P, nb, 4], BF16)
        nc.vector.memset(vT[:], 1.0)
        tp4 = tp[:, 0:4 * nb].rearrange("p (a b) -> p a b", b=4)
        nc.vector.tensor_copy(vT[:, :, 0:3], tp4[:, :, 0:3])

        # numer accumulators: m-chunk j -> bank j//4, partitions 32*(j%4)+0..3
        numer_ps = []
        for bk in range(n_banks):
            nt = npool.tile([P, 512], F32, tag=f"numer{bk}", name=f"numer{bk}")
            nc.vector.memset(nt[:], 0.0)
            numer_ps.append(nt)

        # quadratic Chebyshev fit of exp on [0, 0.25] for the DVE-side exp
        # (energies are >= 0 since q,k are post-relu; observed max ~0.073,
        # fit error ~1e-5 -- far below the bf16 storage rounding of E):
        # exp(x) ~= c2*x^2 + c1*x + c0 = ((x + c1/c2) * x) * c2 + c0
        _xs = np.cos(np.pi * (np.arange(64) + 0.5) / 64) * 0.125 + 0.125
        _cf = np.polyfit(_xs, np.exp(_xs), 2)
        PC2, PC1, PC0 = float(_cf[0]), float(_cf[1]), float(_cf[2])

        # ---- main loop over row blocks ----
        pending = []
        for i in range(nb):
            E_sb = Epool.tile([P, n], BF16, tag="E")
            racc = small.tile([P, max(len(ech), 2)], F32, tag="racc")
            for ci, (off, ln, eng) in enumerate(ech):
                e_ps = epool.tile([P, 1024], F32, tag="e")
                for s in range(0, ln, 512):
                    sl = slice(off + s, off + s + 512)
                    nc.tensor.matmul(e_ps[:, s:s + 512],
                                     q_dr[:, :, i * P:(i + 1) * P],
                                     k_dr[:, :, sl], start=True, stop=True,
                                     perf_mode=mybir.MatmulPerfMode.DoubleRow)
                if eng == "D":
                    nc.vector._custom_dve(
                        EXP_POLY, out=E_sb[:, off:off + ln],
                        in0=e_ps[:, 0:ln], s0=PC1 / PC2, s1=PC2, imm2=PC0,
                        accum_out=racc[:, ci:ci + 1])
                else:
                    nc.scalar.activation(
                        out=E_sb[:, off:off + ln], in_=e_ps[:, 0:ln],
                        func=Exp, accum_out=racc[:, ci:ci + 1])
            rs = small.tile([P, 1], F32, tag="rs")
            nc.vector.reduce_sum(rs[:], racc[:, 0:len(ech)], axis=AX)
            inv = small.tile([P, 1], F32, tag="inv")
            nc.vector.reciprocal(inv[:], rs[:])
            vp = small.tile([P, 4], BF16, tag="vp")
            nc.gpsimd.tensor_scalar_mul(vp[:], vT[:, i, :], inv[:])
            pending.append((vp, E_sb))
            # numer matmuls run one block behind so the PE never starves the
            # ACT/DVE exp of the current block
            if len(pending) > 1:
                pvp, pE = pending.pop(0)
                ip = i - 1
                for j in range(n_mch):
                    jj, bk = j % 4, j // 4
                    nc.tensor.matmul(
                        numer_ps[bk][32 * jj:32 * jj + 4, :], pvp[:],
                        pE[:, j * 512:(j + 1) * 512],
                        start=(ip == 0), stop=False,
                        tile_position=(0, 32 * jj))

        # drain the last pending block's numer matmuls
        pvp, pE = pending.pop(0)
        for j in range(n_mch):
            jj, bk = j % 4, j // 4
            nc.tensor.matmul(
                numer_ps[bk][32 * jj:32 * jj + 4, :], pvp[:],
                pE[:, j * 512:(j + 1) * 512],
                start=(nb == 1), stop=True,
                tile_position=(0, 32 * jj))

        # ---- final: out = alpha * numer/(1e-9+colsum) + x ----
        epsb = consts.tile([P, 1], F32)
        nc.vector.memset(epsb[:], 1e-9)
        qs3 = (nc.gpsimd, nc.sync, nc.scalar)
        for bk in range(n_banks):
            # separate tiles per bank so each bank's chain has no false deps
            recip_b = sb.tile([P, 512], F32, tag=f"recip{bk}", name=f"recip{bk}")
            rep_b = sb.tile([P, 512], F32, tag=f"rep{bk}", name=f"rep{bk}")
            nc.vector.memset(rep_b[:], 0.0)
            nc.scalar.activation(out=recip_b[:], in_=numer_ps[bk][:],
                                 func=Ident, bias=epsb[:], scale=1.0)
            nc.vector.reciprocal(recip_b[:], recip_b[:])
            for jj in range(min(4, n_mch - 4 * bk)):
                srow = recip_b[32 * jj + 3:32 * jj + 4, :]
                # free-dim step-0 broadcast: re-read the same 512 row 4x
                # while the dst walks 4 partitions (partition step 0 is
                # not allowed on SBUF APs)
                src_b = bass.AP(tensor=srow.tensor, offset=srow.offset,
                                ap=[list(srow.ap[0]), [0, 4], list(srow.ap[-1])])
                qs3[jj % 3].dma_start(rep_b[32 * jj:32 * jj + 4, :], src_b)
            att_b = sb.tile([P, 512], F32, tag=f"att{bk}", name=f"att{bk}")
            nc.vector.tensor_mul(att_b[:], numer_ps[bk][:], rep_b[:])
            # out = alpha*att + x in the scattered numer layout (x4 is
            # host-prepared in the same layout), DMA'd straight to DRAM
            out_b = sb.tile([P, 512], F32, tag=f"osc{bk}", name=f"osc{bk}")
            nc.vector.scalar_tensor_tensor(
                out=out_b[:], in0=att_b[:], scalar=alphav[:],
                in1=x4_sb[:, bk, :], op0=AL.mult, op1=AL.add)
            for jj in range(min(4, n_mch - 4 * bk)):
                j = 4 * bk + jj
                qs3[(jj + 1) % 3].dma_start(
                    out_d.ap()[:, j * 512:(j + 1) * 512],
                    out_b[32 * jj:32 * jj + 3, :])

    nc.compile()
    return nc


def fold_weights(inputs):
    """Host-side BN folding. Returns the per-core constant input dict."""
    import ml_dtypes
    bf16 = ml_dtypes.bfloat16

    def fold(w, g, b, m, v):
        s = (g / np.sqrt(v + BN_EPS)).astype(np.float64)
        t = b.astype(np.float64) - s * m.astype(np.float64)
        return s[:, None] * w.astype(np.float64), t

    w1p, t1 = fold(inputs["w1"], inputs["g1"], inputs["b1"],
                   inputs["m1"], inputs["v1"])
    t1 = t1 + float(np.asarray(inputs["offset"]).ravel()[0]) * w1p.sum(axis=1)
    wqp, tq = fold(inputs["wq"], inputs["gq"], inputs["bq"],
                   inputs["mq"], inputs["vq"])
    wkp, tk = fold(inputs["wk"], inputs["gk"], inputs["bk"],
                   inputs["mk"], inputs["vk"])
    wvp, tv = fold(inputs["wv"], inputs["gv"], inputs["bv"],
                   inputs["mv"], inputs["vv"])
    w2 = np.asarray(inputs["w2"]).astype(np.float64)
    wqkv = np.concatenate([wqp, wkp, wvp], axis=0) @ w2   # [35, 64]
    tqkv = np.concatenate([tq, tk, tv], axis=0)           # [35]
    alpha = float(np.asarray(inputs["alpha"]).ravel()[0])
    return {
        "w1t": np.ascontiguousarray(w1p.T).astype(bf16),
        "t1": t1.astype(np.float32).reshape(64, 1),
        "wqkvt": np.ascontiguousarray(wqkv.T).astype(bf16),
        "tqkv": tqkv.astype(np.float32).reshape(35, 1),
        "alphav": np.full((128, 1), alpha, np.float32),
    }


_prog_cache = {}


def get_program(n=N, n_cores=N_CORES):
    key = (n, n_cores)
    if key not in _prog_cache:
        _prog_cache[key] = build_program(n, n_cores)
    return _prog_cache[key]


def make_x4(xb, n=N):
    """Scatter x [3, n] into the numer psum layout [128, n_banks, 512]."""
    n_mch = n // 512
    n_banks = (n_mch + 3) // 4
    x4 = np.zeros((128, n_banks, 512), np.float32)
    for j in range(n_mch):
        jj, bk = j % 4, j // 4
        x4[32 * jj:32 * jj + 3, bk, :] = xb[:, j * 512:(j + 1) * 512]
    return x4


def kernel(_trace=False, _trace_kwargs=None, **inputs):
    import ml_dtypes
    inputs = {k: np.asarray(v) for k, v in inputs.items()}
    nc = get_program()
    const_ins = fold_weights(inputs)
    x = inputs["x"].astype(np.float32)
    in_maps = [dict(const_ins,
                    x4=make_x4(x[b]),
                    xbf=np.ascontiguousarray(x[b]).astype(ml_dtypes.bfloat16))
               for b in range(B)]
    res = run_bass_kernel_spmd(nc, in_maps, core_ids=list(range(N_CORES)),
                               trace=_trace, **(_trace_kwargs or {}))
    out = np.stack([res.results[b]["out"] for b in range(B)], axis=0)
    if _trace:
        kernel.last_result = res
    return out.astype(np.float32)


if __name__ == "__main__":
    t0 = time.time()
    nc = get_program()
    print("build+compile:", time.time() - t0, flush=True)
